# revision 4
# baseline (speedup 1.0000x reference)
"""Trainium2 Bass kernel for nn_DotProcessorBlock.

Computes, for x:[B,N] f32 (B=4096, N=256), w,b:[N]:
    feat = x * w + b                      (elementwise affine on features)
    Z[b,i,j] = feat[b,i] * feat[b,j]      (batched outer product)
    out = Z.reshape(B, N*N)[:, :N*(N+1)//2]   -> [4096, 32896]

Sharding: data-parallel batch split across 8 NeuronCores (512 rows each);
w/b replicated. Output is ~539 MB f32 so the kernel is bound by the HBM
output-write bandwidth (~67 MB/core -> ~190us at ~360 GB/s).

Per-core layout: batch rows in SBUF partitions. For a 128-row batch tile,
out[b, i*256+j] = feat[b,i]*feat[b,j] is produced one i at a time as a
per-partition-scalar broadcast multiply (DVE tensor_scalar at 2x fp32 mode,
with a share on ACT via activation-copy-with-scale). Chunks of 32 i-values
(32KB/partition) stream to HBM as ~4.2MB DMAs on the SP HWDGE ring.

Columns 32768:32896 ("i=128, j<128" of the truncated flatten) are
feat[b,128]*feat[b,j], j<128 — one extra [128,128] tensor_scalar folded
into the last chunk's DMA.
"""

from contextlib import ExitStack

import numpy as np

import concourse.bacc as bacc
import concourse.bass as bass
import concourse.tile as tile
from concourse import mybir
from concourse.bass_utils import run_bass_kernel_spmd

B_FULL = 4096
N = 256
N_CORES = 8
B_CORE = B_FULL // N_CORES          # 512
NUM_INTS = N * (N + 1) // 2         # 32896
P = 128                             # SBUF partitions = batch rows per tile
N_BT = B_CORE // P                  # 4 batch tiles per core
CH_I = 32                           # i-values per output chunk
N_CH = P // CH_I                    # 4 chunks of full-width i rows
CHUNK = CH_I * N                    # 8192 columns per chunk
TAIL = P                            # 128 tail columns (i=128, j<128)

FP32 = mybir.dt.float32


def _emit(ctx, tc, out, x, wb, bb):
    nc = tc.nc
    const_pool = ctx.enter_context(tc.tile_pool(name="const", bufs=1))
    x_pool = ctx.enter_context(tc.tile_pool(name="x", bufs=2))
    f_pool = ctx.enter_context(tc.tile_pool(name="feat", bufs=2))
    o_pool = ctx.enter_context(tc.tile_pool(name="out", bufs=4))

    w_t = const_pool.tile([P, N], FP32, tag="w")
    b_t = const_pool.tile([P, N], FP32, tag="b")
    nc.sync.dma_start(w_t[:], wb[:])
    nc.sync.dma_start(b_t[:], bb[:])

    for bt in range(N_BT):
        x_t = x_pool.tile([P, N], FP32, tag="x")
        nc.sync.dma_start(x_t[:], x[bt * P:(bt + 1) * P, :])

        feat = f_pool.tile([P, N], FP32, tag="feat")
        nc.vector.tensor_mul(feat[:], x_t[:], w_t[:])
        nc.vector.tensor_add(feat[:], feat[:], b_t[:])

        for c in range(N_CH):
            ot = o_pool.tile([P, CHUNK + TAIL], FP32, tag="ot")
            for k in range(CH_I):
                i = c * CH_I + k
                dst = ot[:, k * N:(k + 1) * N]
                if i % 4 == 3:
                    # ACT: out = Copy(in * scale), scale = per-partition feat[:,i]
                    nc.scalar.mul(dst, feat[:], feat[:, i:i + 1])
                else:
                    # DVE: fp32 tensor_scalar runs in 2x_2P mode
                    nc.vector.tensor_scalar_mul(dst, feat[:], feat[:, i:i + 1])
            sz = CHUNK
            if c == N_CH - 1:
                nc.vector.tensor_scalar_mul(
                    ot[:, CHUNK:CHUNK + TAIL], feat[:, 0:TAIL], feat[:, P:P + 1]
                )
                sz = CHUNK + TAIL
            nc.sync.dma_start(
                out[bt * P:(bt + 1) * P, c * CHUNK:c * CHUNK + sz], ot[:, :sz]
            )


def _build():
    nc = bacc.Bacc("TRN2", target_bir_lowering=False, debug=False,
                   num_devices=N_CORES)
    x = nc.dram_tensor("x", [B_CORE, N], FP32, kind="ExternalInput").ap()
    wb = nc.dram_tensor("weight_w", [P, N], FP32, kind="ExternalInput").ap()
    bb = nc.dram_tensor("weight_b", [P, N], FP32, kind="ExternalInput").ap()
    out = nc.dram_tensor("out", [B_CORE, NUM_INTS], FP32,
                         kind="ExternalOutput").ap()
    with tile.TileContext(nc) as tc, ExitStack() as ctx:
        _emit(ctx, tc, out, x, wb, bb)
    nc.compile()
    return nc


_NC_CACHE = None


def _get_nc():
    global _NC_CACHE
    if _NC_CACHE is None:
        _NC_CACHE = _build()
    return _NC_CACHE


def run(x, weight_w, weight_b, trace=False, **run_kwargs):
    x = np.ascontiguousarray(np.asarray(x, dtype=np.float32))
    w = np.asarray(weight_w, dtype=np.float32).reshape(N)
    b = np.asarray(weight_b, dtype=np.float32).reshape(N)
    assert x.shape == (B_FULL, N), x.shape

    wb = np.ascontiguousarray(np.broadcast_to(w, (P, N)))
    bb = np.ascontiguousarray(np.broadcast_to(b, (P, N)))
    in_maps = [
        {"x": x[i * B_CORE:(i + 1) * B_CORE], "weight_w": wb, "weight_b": bb}
        for i in range(N_CORES)
    ]
    res = run_bass_kernel_spmd(
        _get_nc(), in_maps, core_ids=list(range(N_CORES)), trace=trace,
        **run_kwargs,
    )
    full = np.concatenate([r["out"] for r in res.results], axis=0)
    return full, res


def kernel(x, weight_w, weight_b):
    full, _ = run(x, weight_w, weight_b, trace=False)
    return full


# revision 6
# speedup vs baseline: 1.0213x; 1.0213x over previous
"""Trainium2 Bass kernel for nn_DotProcessorBlock.

Computes, for x:[B,N] f32 (B=4096, N=256), w,b:[N]:
    feat = x * w + b                      (elementwise affine on features)
    Z[b,i,j] = feat[b,i] * feat[b,j]      (batched outer product)
    out = Z.reshape(B, N*N)[:, :N*(N+1)//2]   -> [4096, 32896]

Sharding: data-parallel batch split across 8 NeuronCores (512 rows each);
w/b replicated. Output is ~539 MB f32 so the kernel is bound by the HBM
output-write bandwidth (~67 MB/core -> ~190us at ~360 GB/s).

Per-core layout: batch rows in SBUF partitions. For a 128-row batch tile,
out[b, i*256+j] = feat[b,i]*feat[b,j] is produced one i at a time as a
per-partition-scalar broadcast multiply (DVE tensor_scalar at 2x fp32 mode,
with a share on ACT via activation-copy-with-scale). Chunks of 32 i-values
(32KB/partition) stream to HBM as ~4.2MB DMAs on the SP HWDGE ring.

Columns 32768:32896 ("i=128, j<128" of the truncated flatten) are
feat[b,128]*feat[b,j], j<128 — one extra [128,128] tensor_scalar folded
into the last chunk's DMA.
"""

from contextlib import ExitStack

import numpy as np

import concourse.bacc as bacc
import concourse.bass as bass
import concourse.tile as tile
from concourse import mybir
from concourse.bass_utils import run_bass_kernel_spmd

B_FULL = 4096
N = 256
N_CORES = 8
B_CORE = B_FULL // N_CORES          # 512
NUM_INTS = N * (N + 1) // 2         # 32896
P = 128                             # SBUF partitions = batch rows per tile
N_BT = B_CORE // P                  # 4 batch tiles per core
CH_I = 32                           # i-values per output chunk
N_CH = P // CH_I                    # 4 chunks of full-width i rows
CHUNK = CH_I * N                    # 8192 columns per chunk
TAIL = P                            # 128 tail columns (i=128, j<128)

FP32 = mybir.dt.float32


# Per-batch-tile chunk schedule: (n_i, act_share) pairs summing to 128
# i-values. Small chunks at the kernel's start (fast pipeline fill) and end
# (short final drain); 32-wide chunks (4.2 MB DMAs) in the middle. act_share
# i-values go to ACT as per-i activation-copy ops; the rest are covered by a
# single stride-0-broadcast tensor_tensor on DVE (~0.28 ns/col/partition vs
# ACT's ~2.3), balancing the two engines at roughly 22/10 per 32.
_MID = [(32, 10), (32, 10), (32, 10), (32, 10)]
_SCHED = {
    0: [(8, 2), (24, 7), (32, 10), (32, 10), (32, 10)],
    N_BT - 1: [(32, 10), (32, 10), (32, 10), (24, 7), (8, 2)],
}


def _emit_chunk(nc, feat, ot, c0, n_i, act_share, with_tail):
    d = n_i - act_share
    if d > 0:
        out3 = ot[:, 0:d * N].rearrange("p (a b) -> p a b", a=d, b=N)
        in0 = feat[:].unsqueeze(1).broadcast_to((P, d, N))
        in1 = feat[:, c0:c0 + d].unsqueeze(2).broadcast_to((P, d, N))
        nc.vector.tensor_mul(out3, in0, in1)
    for k in range(d, n_i):
        nc.scalar.mul(ot[:, k * N:(k + 1) * N], feat[:], feat[:, c0 + k:c0 + k + 1])
    if with_tail:
        nc.vector.tensor_scalar_mul(
            ot[:, n_i * N:n_i * N + TAIL], feat[:, 0:TAIL], feat[:, P:P + 1]
        )


def _emit(ctx, tc, out, x, wb, bb):
    nc = tc.nc
    const_pool = ctx.enter_context(tc.tile_pool(name="const", bufs=1))
    x_pool = ctx.enter_context(tc.tile_pool(name="x", bufs=4))
    f_pool = ctx.enter_context(tc.tile_pool(name="feat", bufs=4))
    o_pool = ctx.enter_context(tc.tile_pool(name="out", bufs=5))

    w_t = const_pool.tile([P, N], FP32, tag="w")
    b_t = const_pool.tile([P, N], FP32, tag="b")
    nc.sync.dma_start(w_t[:], wb[:])
    nc.sync.dma_start(b_t[:], bb[:])

    for bt in range(N_BT):
        x_t = x_pool.tile([P, N], FP32, tag="x")
        nc.sync.dma_start(x_t[:], x[bt * P:(bt + 1) * P, :])

        feat = f_pool.tile([P, N], FP32, tag="feat")
        nc.vector.tensor_mul(feat[:], x_t[:], w_t[:])
        nc.vector.tensor_add(feat[:], feat[:], b_t[:])

        c0 = 0
        sched = _SCHED.get(bt, _MID)
        for ci, (n_i, act_share) in enumerate(sched):
            last = ci == len(sched) - 1  # tail cols are per-row: every bt
            sz = n_i * N + (TAIL if last else 0)
            ot = o_pool.tile([P, sz], FP32, tag="ot")
            _emit_chunk(nc, feat, ot, c0, n_i, act_share, last)
            nc.sync.dma_start(
                out[bt * P:(bt + 1) * P, c0 * N:c0 * N + sz], ot[:, :sz]
            )
            c0 += n_i


def _build():
    nc = bacc.Bacc("TRN2", target_bir_lowering=False, debug=False,
                   num_devices=N_CORES)
    x = nc.dram_tensor("x", [B_CORE, N], FP32, kind="ExternalInput").ap()
    wb = nc.dram_tensor("weight_w", [P, N], FP32, kind="ExternalInput").ap()
    bb = nc.dram_tensor("weight_b", [P, N], FP32, kind="ExternalInput").ap()
    out = nc.dram_tensor("out", [B_CORE, NUM_INTS], FP32,
                         kind="ExternalOutput").ap()
    with tile.TileContext(nc) as tc, ExitStack() as ctx:
        _emit(ctx, tc, out, x, wb, bb)
    nc.compile()
    return nc


_NC_CACHE = None


def _get_nc():
    global _NC_CACHE
    if _NC_CACHE is None:
        _NC_CACHE = _build()
    return _NC_CACHE


def run(x, weight_w, weight_b, trace=False, **run_kwargs):
    x = np.ascontiguousarray(np.asarray(x, dtype=np.float32))
    w = np.asarray(weight_w, dtype=np.float32).reshape(N)
    b = np.asarray(weight_b, dtype=np.float32).reshape(N)
    assert x.shape == (B_FULL, N), x.shape

    wb = np.ascontiguousarray(np.broadcast_to(w, (P, N)))
    bb = np.ascontiguousarray(np.broadcast_to(b, (P, N)))
    in_maps = [
        {"x": x[i * B_CORE:(i + 1) * B_CORE], "weight_w": wb, "weight_b": bb}
        for i in range(N_CORES)
    ]
    res = run_bass_kernel_spmd(
        _get_nc(), in_maps, core_ids=list(range(N_CORES)), trace=trace,
        **run_kwargs,
    )
    full = np.concatenate([r["out"] for r in res.results], axis=0)
    return full, res


def kernel(x, weight_w, weight_b):
    full, _ = run(x, weight_w, weight_b, trace=False)
    return full


# revision 16
# speedup vs baseline: 1.0291x; 1.0076x over previous
"""Trainium2 Bass kernel for nn_DotProcessorBlock.

Computes, for x:[B,N] f32 (B=4096, N=256), w,b:[N]:
    feat = x * w + b                      (elementwise affine on features)
    Z[b,i,j] = feat[b,i] * feat[b,j]      (batched outer product)
    out = Z.reshape(B, N*N)[:, :N*(N+1)//2]   -> [4096, 32896]

Sharding: data-parallel batch split across 8 NeuronCores (512 rows each);
w/b replicated. Output is ~539 MB f32 so the kernel is bound by the HBM
output-write bandwidth (~67 MB/core -> ~190us at ~360 GB/s).

Per-core layout: batch rows in SBUF partitions. For a 128-row batch tile,
out[b, i*256+j] = feat[b,i]*feat[b,j] is produced in chunks of i-values:
one stride-0-broadcast fp32 tensor_tensor on DVE covers ~22 of every 32
i-values (in0 = feat broadcast over i, in1 = feat[:, i-range] broadcast
over j; 1 elem/lane/cycle, ~6us per instruction), the rest go to ACT as
per-i activation-copy-with-scale ops, balancing the two engines. Chunks
(32KB/partition) stream to HBM as ~4.2MB DMAs on the SP HWDGE ring, which
sustains ~423 GB/s — the kernel is DMA-write-bound (~161us of DMA active
time; ~175-181us/core total, more when the paired NeuronCore contends for
the shared HBM stack).

Columns 32768:32896 ("i=128, j<128" of the truncated flatten) are
feat[b,128]*feat[b,j], j<128 — one extra [128,128] tensor_scalar folded
into each batch tile's last chunk DMA.
"""

from contextlib import ExitStack

import numpy as np

import concourse.bacc as bacc
import concourse.bass as bass
import concourse.tile as tile
from concourse import mybir
from concourse.bass_utils import run_bass_kernel_spmd
from concourse.tile_rust import add_dep_helper

B_FULL = 4096
N = 256
N_CORES = 8
B_CORE = B_FULL // N_CORES          # 512
NUM_INTS = N * (N + 1) // 2         # 32896
P = 128                             # SBUF partitions = batch rows per tile
N_BT = B_CORE // P                  # 4 batch tiles per core
CH_I = 32                           # i-values per output chunk
N_CH = P // CH_I                    # 4 chunks of full-width i rows
CHUNK = CH_I * N                    # 8192 columns per chunk
TAIL = P                            # 128 tail columns (i=128, j<128)

FP32 = mybir.dt.float32


# Per-batch-tile chunk schedule: (n_i, act_share) pairs summing to 128
# i-values. Tiny leading chunks on bt0 get the output-DMA stream started as
# early as possible; 32-wide chunks (4.2 MB DMAs) elsewhere. act_share
# i-values go to ACT as per-i activation-copy ops; the rest are covered by a
# single stride-0-broadcast tensor_tensor on DVE, balancing the two engines.
_MID = [(32, 10), (32, 10), (32, 10), (32, 10)]
_SCHED = {
    0: [(4, 1), (12, 3), (24, 7), (32, 10), (32, 10), (24, 8)],
}


def _emit_chunk(nc, feat, ot, c0, n_i, act_share, with_tail):
    d = n_i - act_share
    tt_inst = None
    if d > 0:
        out3 = ot[:, 0:d * N].rearrange("p (a b) -> p a b", a=d, b=N)
        in0 = feat[:].unsqueeze(1).broadcast_to((P, d, N))
        in1 = feat[:, c0:c0 + d].unsqueeze(2).broadcast_to((P, d, N))
        tt_inst = nc.vector.tensor_mul(out3, in0, in1)
    for k in range(d, n_i):
        nc.scalar.mul(ot[:, k * N:(k + 1) * N], feat[:], feat[:, c0 + k:c0 + k + 1])
    if with_tail:
        nc.vector.tensor_scalar_mul(
            ot[:, n_i * N:n_i * N + TAIL], feat[:, 0:TAIL], feat[:, P:P + 1]
        )
    return tt_inst


def _emit(ctx, tc, out, x, wb):
    nc = tc.nc
    const_pool = ctx.enter_context(tc.tile_pool(name="const", bufs=1))
    x_pool = ctx.enter_context(tc.tile_pool(name="x", bufs=4))
    f_pool = ctx.enter_context(tc.tile_pool(name="feat", bufs=4))
    o_pool = ctx.enter_context(tc.tile_pool(name="out", bufs=5))

    # wb + x0 issue first on the otherwise-idle SP ring (ACT is busy with
    # its table load at kernel start); later x tiles load via the ACT ring
    # so SP carries only the output stream after the first chunk.
    wb_t = const_pool.tile([P, 2 * N], FP32, tag="wb")
    nc.sync.dma_start(wb_t[:], wb[:])
    w_t = wb_t[:, 0:N]
    b_t = wb_t[:, N:2 * N]

    def load_feat(bt, order_after=None):
        x_t = x_pool.tile([P, N], FP32, tag="x")
        x_dma_eng = nc.sync if bt == 0 else nc.scalar
        x_dma_eng.dma_start(x_t[:], x[bt * P:(bt + 1) * P, :])
        feat = f_pool.tile([P, N], FP32, tag="feat")
        mul = nc.vector.tensor_mul(feat[:], x_t[:], w_t)
        if order_after is not None:
            # Order-only edge: keep the next feat's DVE ops from being
            # statically scheduled ahead of the fill-critical first chunks.
            add_dep_helper(mul.ins, order_after.ins, sync=False,
                           reason="fill path first on DVE")
        nc.vector.tensor_add(feat[:], feat[:], b_t)
        return feat

    feat = load_feat(0)
    for bt in range(N_BT):
        c0 = 0
        sched = _SCHED.get(bt, _MID)
        next_feat = None
        for ci, (n_i, act_share) in enumerate(sched):
            last = ci == len(sched) - 1  # tail cols are per-row: every bt
            sz = n_i * N + (TAIL if last else 0)
            ot = o_pool.tile([P, sz], FP32, tag="ot")
            tt = _emit_chunk(nc, feat, ot, c0, n_i, act_share, last)
            nc.sync.dma_start(
                out[bt * P:(bt + 1) * P, c0 * N:c0 * N + sz], ot[:, :sz]
            )
            c0 += n_i
            # Emit the next batch-tile's load+feat after this tile's second
            # chunk, ordered behind it on DVE.
            if ci == 1 and bt + 1 < N_BT:
                next_feat = load_feat(bt + 1, order_after=tt)
        feat = next_feat


def _build():
    nc = bacc.Bacc("TRN2", target_bir_lowering=False, debug=False,
                   num_devices=N_CORES)
    x = nc.dram_tensor("x", [B_CORE, N], FP32, kind="ExternalInput").ap()
    wb = nc.dram_tensor("wb", [P, 2 * N], FP32, kind="ExternalInput").ap()
    out = nc.dram_tensor("out", [B_CORE, NUM_INTS], FP32,
                         kind="ExternalOutput").ap()
    with tile.TileContext(nc) as tc, ExitStack() as ctx:
        _emit(ctx, tc, out, x, wb)
    nc.compile()
    return nc


_NC_CACHE = None


def _get_nc():
    global _NC_CACHE
    if _NC_CACHE is None:
        _NC_CACHE = _build()
    return _NC_CACHE


def run(x, weight_w, weight_b, trace=False, **run_kwargs):
    x = np.ascontiguousarray(np.asarray(x, dtype=np.float32))
    w = np.asarray(weight_w, dtype=np.float32).reshape(N)
    b = np.asarray(weight_b, dtype=np.float32).reshape(N)
    assert x.shape == (B_FULL, N), x.shape

    wb = np.ascontiguousarray(
        np.broadcast_to(np.concatenate([w, b]), (P, 2 * N))
    )
    in_maps = [
        {"x": x[i * B_CORE:(i + 1) * B_CORE], "wb": wb}
        for i in range(N_CORES)
    ]
    res = run_bass_kernel_spmd(
        _get_nc(), in_maps, core_ids=list(range(N_CORES)), trace=trace,
        **run_kwargs,
    )
    full = np.concatenate([r["out"] for r in res.results], axis=0)
    return full, res


def kernel(x, weight_w, weight_b):
    full, _ = run(x, weight_w, weight_b, trace=False)
    return full
